# revision 13
# baseline (speedup 1.0000x reference)
"""MoE decoder block (nn_MoEDecoderBlock_78855599554928) on 8 TRN2 NeuronCores.

Sharding: token-parallel attention (no collective; causal work made uniform by
computing k/v for all rows and masking with per-core 0/1/triangular mask data;
core c owns query blocks {batch0 block c, batch1 block 7-c}), expert-parallel
MoE (core c owns routed expert c + a 1376-wide slice of the shared FFN).
Cross-core traffic: one AllGather of the FFN-normed activations (hnT,
[D, 256] f32 per core) and one ReduceScatter of the partial MoE output
([2048, 2048] f32). The SPMD program is identical on all cores; per-core
differences are carried entirely in input data.

Matmuls run as float32r (full PE rate at moving-dim>=256) except the
score/attention-value matmuls which run bf16 (conversion is free in the
PSUM->SBUF copies). Softmax skips max-subtraction (scores are O(1) here);
the causal mask is a 0/1 multiply on exp values; the denominator comes from
a ones-vector matmul.

Self-contained: hardcodes all shapes from the problem spec.
"""

import math

import numpy as np

import concourse.bass as bass
import concourse.mybir as mybir
import concourse.tile as tile

B = 2
S = 1024
D = 2048
HQ = 16
HKV = 4
HD = 128
E = 8
F = 2048
FS = 11008
FSS = FS // 8  # 1376 shared-FFN cols per core
TOPK = 2
EPS = 1e-6
NCORES = 8
T = B * S  # 2048
TOWN = T // NCORES  # 256
DC = D // 128  # 16
FC = F // 128  # 16
FSC = [128] * (FSS // 128) + ([FSS % 128] if FSS % 128 else [])

F32 = mybir.dt.float32
F32R = mybir.dt.float32r
BF16 = mybir.dt.bfloat16
MUL = mybir.AluOpType.mult
ADD = mybir.AluOpType.add
SUB = mybir.AluOpType.subtract

ISR = 1.0 / math.sqrt(HD)


def _mm(nc, out, lhsT, rhs, start, stop):
    nc.tensor.matmul(out, lhsT, rhs, start=start, stop=stop)


def _emit_rope(nc, pool, dst, src, cos, sin, L):
    """Rope in [HD, rows] layout. src: [128, L] (PSUM f32); cos/sin: SBUF
    [128, L] f32; dst: SBUF [128, L] (any dtype)."""
    lo = slice(0, 64)
    hi = slice(64, 128)
    tmp = pool.tile([128, L], F32, tag="rope_tmp")
    t2 = pool.tile([128, L], F32, tag="rope_tmp2")
    nc.vector.tensor_tensor(tmp[lo, :], src[hi, :], sin[lo, :], MUL)
    nc.vector.tensor_tensor(tmp[hi, :], src[lo, :], sin[hi, :], MUL)
    nc.vector.tensor_tensor(t2[lo, :], src[lo, :], cos[lo, :], MUL)
    nc.vector.tensor_tensor(t2[hi, :], src[hi, :], cos[hi, :], MUL)
    nc.vector.tensor_tensor(dst[lo, :], t2[lo, :], tmp[lo, :], SUB)
    nc.vector.tensor_tensor(dst[hi, :], t2[hi, :], tmp[hi, :], ADD)


def _emit_normT(nc, pool, psp, dst, stride, src_dram, row_off, n_rows,
                w_col, ones_col_f, ones_row_f, eps1):
    """rmsnorm in transposed layout, in place (dst: SBUF f32r tile with
    chunk dc at free offset dc*stride)."""
    AOT = mybir.ActivationFunctionType
    for dc in range(DC):
        nc.sync.dma_start(
            out=dst[:, dc * stride : dc * stride + n_rows],
            in_=src_dram[dc * 128 : (dc + 1) * 128, row_off : row_off + n_rows],
        )
    ps_ss = psp.tile([1, n_rows], F32, tag="ps_ss")
    for dc in range(DC):
        sq = pool.tile([128, n_rows], F32R, tag="nrm_sq")
        sl = slice(dc * stride, dc * stride + n_rows)
        nc.vector.tensor_tensor(sq[:], dst[:, sl], dst[:, sl], MUL)
        _mm(nc, ps_ss[:], ones_col_f[:], sq[:], dc == 0, dc == DC - 1)
    srow = pool.tile([1, n_rows], F32, tag="nrm_s")
    nc.scalar.activation(srow[:], ps_ss[:], AOT.Sqrt, scale=1.0 / D, bias=eps1[:])
    rrow = pool.tile([1, n_rows], F32R, tag="nrm_r")
    with nc.allow_low_precision(reason="f32r-typed copy for PE broadcast rhs"):
        nc.vector.reciprocal(rrow[:], srow[:])
    ps_bc = psp.tile([128, n_rows], F32, tag="ps_bc")
    _mm(nc, ps_bc[:], ones_row_f[:], rrow[:], True, True)
    for dc in range(DC):
        sl = slice(dc * stride, dc * stride + n_rows)
        nc.vector.scalar_tensor_tensor(
            dst[:, sl], dst[:, sl], w_col[:, dc : dc + 1], ps_bc[:], MUL, MUL)


def build_bass(repeats=1):
    nc = bass.Bass(
        "TRN2", target_bir_lowering=False, debug=False, num_devices=NCORES
    )

    io = {}
    for name, shape, dt in [
        ("xT", [D, T], F32R), ("cosT", [HD, T], F32), ("sinT", [HD, T], F32),
        ("attn_w", [D, 1], F32), ("ffn_w", [D, 1], F32),
        ("wq", [D, HQ * HD], F32R), ("wk", [D, HKV * HD], F32R),
        ("wv", [D, HKV * HD], F32R), ("wo", [HQ * HD, D], F32R),
        ("router_w", [D, E], F32R), ("identity", [128, 128], F32),
        ("x_own", [TOWN, D], F32), ("xT_own", [D, TOWN], F32R),
        ("cosT_own", [HD, TOWN], F32), ("sinT_own", [HD, TOWN], F32),
        ("amask", [16 * 128, 128], BF16), ("esel", [128, E], F32),
        ("wg", [D, F], F32R), ("wu", [D, F], F32R), ("wd", [F, D], F32R),
        ("wsg", [D, FSS], F32R), ("wsu", [D, FSS], F32R),
        ("wsd", [FSS, D], F32R),
    ]:
        io[name] = nc.dram_tensor(name, shape, dt, kind="ExternalInput")
    for name, shape in [("out_sh", [TOWN, D]), ("newk", [TOWN, HQ * HD]),
                        ("newv", [TOWN, HQ * HD]), ("aux", [1, 1])]:
        io[name] = nc.dram_tensor(name, shape, F32, kind="ExternalOutput")

    with tile.TileContext(nc) as tc:
        with (
            tc.tile_pool(name="const", bufs=1) as constp,
            tc.tile_pool(name="dram", bufs=1, space="DRAM") as dramp,
            tc.tile_pool(name="bigB", bufs=1) as bigB,
        ):

            cst = {}
            cst["fwt"] = constp.tile([128, DC], F32, name="fwt")
            nc.sync.dma_start(
                out=cst["fwt"][:],
                in_=io["ffn_w"].ap().rearrange("(c p) one -> p (c one)", p=128))
            cst["eselt"] = constp.tile([128, E], F32, name="eselt")
            nc.sync.dma_start(out=cst["eselt"][:], in_=io["esel"][:])
            cst["identt"] = constp.tile([128, 128], F32, name="identt")
            nc.sync.dma_start(out=cst["identt"][:], in_=io["identity"][:])
            for nm, dt, val, shape in [
                ("ones_col_b", BF16, 1.0, [128, 1]),
                ("ones_row_b", BF16, 1.0, [1, 128]),
                ("ones_col_f", F32R, 1.0, [128, 1]),
                ("ones_row_f", F32R, 1.0, [1, 128]),
                ("epsc", F32, EPS, [128, 1]),
                ("eps1", F32, EPS, [1, 1]),
            ]:
                cst[nm] = constp.tile(shape, dt, name=nm)
                if dt == F32R:
                    scr = constp.tile(shape, F32, name=nm + "_f32src")
                    nc.gpsimd.memset(scr[:], val)
                    nc.vector.tensor_copy(cst[nm][:], scr[:])
                else:
                    nc.gpsimd.memset(cst[nm][:], val)

            big = dict(
                oTh=bigB.tile([128, HQ * TOWN], F32R, name="oTh"),
                xloc=bigB.tile([128, 2 * D], F32, name="xloc"),
            )
            for _rep in range(repeats):
                dram = dict(
                    ag_in=dramp.tile([D, TOWN], F32, name=f"ag_in{_rep}"),
                    ag_out=dramp.tile([NCORES * D, TOWN], F32,
                                      name=f"ag_out{_rep}", addr_space="Shared"),
                    rs_in=dramp.tile([T, D], F32, name=f"rs_in{_rep}"),
                    rs_out=dramp.tile([TOWN, D], F32, name=f"rs_out{_rep}"),
                )
                _emit_body(nc, tc, io, dram, cst, big)

    _split_multi_waits(nc)
    return nc


def _emit_body(nc, tc, io, dram, cst, big):
    AOT = mybir.ActivationFunctionType
    ones_col_f = cst["ones_col_f"]
    ones_row_f = cst["ones_row_f"]
    oTh = big["oTh"]
    xloc = big["xloc"]

    with tc.tile_pool(name="bigA", bufs=1) as bigA:
        kv_kT = bigA.tile([128, HKV * T], BF16)
        kv_v = bigA.tile([128, 16 * 512], BF16)
        qT = bigA.tile([128, HQ * TOWN], BF16)
        cost = bigA.tile([128, T], F32)
        nc.sync.dma_start(out=cost[:], in_=io["cosT"][:])
        sint = bigA.tile([128, T], F32)
        nc.sync.dma_start(out=sint[:], in_=io["sinT"][:])
        cosot = bigA.tile([128, TOWN], F32)
        nc.sync.dma_start(out=cosot[:], in_=io["cosT_own"][:])
        sinot = bigA.tile([128, TOWN], F32)
        nc.sync.dma_start(out=sinot[:], in_=io["sinT_own"][:])
        maskt = bigA.tile([128, 16 * 128], BF16)
        for i in range(16):
            nc.sync.dma_start(out=maskt[:, i * 128 : (i + 1) * 128],
                              in_=io["amask"][i * 128 : (i + 1) * 128, :])
        awt = bigA.tile([128, DC], F32)
        nc.sync.dma_start(
            out=awt[:],
            in_=io["attn_w"].ap().rearrange("(c p) one -> p (c one)", p=128))

        # ===== stage 1: own tokens -> roped q, new_k, new_v =====
        with (
            tc.tile_pool(name="s1", bufs=2) as s1,
            tc.tile_pool(name="s1w", bufs=2) as s1w,
            tc.tile_pool(name="s1ps", bufs=2, space="PSUM") as s1ps,
        ):
            hq = s1.tile([128, DC * TOWN], F32R, tag="hq", bufs=1)
            _emit_normT(nc, s1, s1ps, hq, TOWN, io["xT_own"], 0, TOWN,
                        awt, ones_col_f, ones_row_f, cst["eps1"])
            wvt = s1w.tile([128, DC * 512], F32R, tag="wvt", bufs=1)
            for dc in range(DC):
                nc.sync.dma_start(out=wvt[:, dc * 512 : (dc + 1) * 512],
                                  in_=io["wv"][dc * 128 : (dc + 1) * 128, :])
            for h in range(HQ):
                wqt = s1w.tile([128, DC * 128], F32R, tag="wqt")
                for dc in range(DC):
                    nc.sync.dma_start(
                        out=wqt[:, dc * 128 : (dc + 1) * 128],
                        in_=io["wq"][dc * 128 : (dc + 1) * 128,
                                     h * 128 : (h + 1) * 128])
                ps_q = s1ps.tile([128, TOWN], F32, tag="ps_q")
                for dc in range(DC):
                    _mm(nc, ps_q[:], wqt[:, dc * 128 : (dc + 1) * 128],
                        hq[:, dc * TOWN : (dc + 1) * TOWN], dc == 0, dc == DC - 1)
                _emit_rope(nc, s1, qT[:, h * TOWN : (h + 1) * TOWN], ps_q[:],
                           cosot, sinot, TOWN)
            for g in range(HKV):
                wkt = s1w.tile([128, DC * 128], F32R, tag="wqt")
                for dc in range(DC):
                    nc.sync.dma_start(
                        out=wkt[:, dc * 128 : (dc + 1) * 128],
                        in_=io["wk"][dc * 128 : (dc + 1) * 128,
                                     g * 128 : (g + 1) * 128])
                ps_k = s1ps.tile([128, TOWN], F32, tag="ps_q")
                for dc in range(DC):
                    _mm(nc, ps_k[:], wkt[:, dc * 128 : (dc + 1) * 128],
                        hq[:, dc * TOWN : (dc + 1) * TOWN], dc == 0, dc == DC - 1)
                ko = s1.tile([128, TOWN], F32, tag="ko")
                _emit_rope(nc, s1, ko[:], ps_k[:], cosot, sinot, TOWN)
                for r in range(2):
                    ps_t = s1ps.tile([128, 128], F32, tag="ps_t")
                    nc.tensor.transpose(
                        ps_t[:], ko[:, r * 128 : (r + 1) * 128], cst["identt"][:])
                    kn = s1.tile([128, 128], F32, tag="kn")
                    nc.scalar.copy(kn[:], ps_t[:])
                    for j in range(4):
                        h = g * 4 + j
                        nc.sync.dma_start(
                            out=io["newk"][r * 128 : (r + 1) * 128,
                                           h * 128 : (h + 1) * 128],
                            in_=kn[:])
            for r in range(2):
                ps_v = s1ps.tile([128, 512], F32, tag="ps_q")
                for dc in range(DC):
                    _mm(nc, ps_v[:],
                        hq[:, dc * TOWN + r * 128 : dc * TOWN + (r + 1) * 128],
                        wvt[:, dc * 512 : (dc + 1) * 512], dc == 0, dc == DC - 1)
                vn = s1.tile([128, 512], F32, tag="vn")
                nc.scalar.copy(vn[:], ps_v[:])
                for g in range(HKV):
                    for j in range(4):
                        h = g * 4 + j
                        nc.sync.dma_start(
                            out=io["newv"][r * 128 : (r + 1) * 128,
                                           h * 128 : (h + 1) * 128],
                            in_=vn[:, g * 128 : (g + 1) * 128])

        # ===== stage 2: k/v for all 2048 rows =====
        with (
            tc.tile_pool(name="s2", bufs=2) as s2,
            tc.tile_pool(name="s2h", bufs=1) as s2h,
            tc.tile_pool(name="s2ps", bufs=2, space="PSUM") as s2ps,
        ):
            wvt = s2h.tile([128, DC * 512], F32R, tag="wvt", bufs=1)
            for dc in range(DC):
                nc.sync.dma_start(out=wvt[:, dc * 512 : (dc + 1) * 512],
                                  in_=io["wv"][dc * 128 : (dc + 1) * 128, :])
            for grp in range(4):
                hT = s2h.tile([128, DC * 512], F32R, tag="hT", bufs=1)
                _emit_normT(nc, s2, s2ps, hT, 512, io["xT"], grp * 512, 512,
                            awt, ones_col_f, ones_row_f, cst["eps1"])
                for g in range(HKV):
                    wkt = s2.tile([128, DC * 128], F32R, tag="wkt")
                    for dc in range(DC):
                        nc.sync.dma_start(
                            out=wkt[:, dc * 128 : (dc + 1) * 128],
                            in_=io["wk"][dc * 128 : (dc + 1) * 128,
                                         g * 128 : (g + 1) * 128])
                    ps_k = s2ps.tile([128, 512], F32, tag="ps_k")
                    for dc in range(DC):
                        _mm(nc, ps_k[:], wkt[:, dc * 128 : (dc + 1) * 128],
                            hT[:, dc * 512 : (dc + 1) * 512], dc == 0, dc == DC - 1)
                    _emit_rope(
                        nc, s2,
                        kv_kT[:, g * T + grp * 512 : g * T + (grp + 1) * 512],
                        ps_k[:], cost[:, grp * 512 : (grp + 1) * 512],
                        sint[:, grp * 512 : (grp + 1) * 512], 512)
                for r in range(4):
                    ps_v = s2ps.tile([128, 512], F32, tag="ps_k")
                    for dc in range(DC):
                        _mm(nc, ps_v[:],
                            hT[:, dc * 512 + r * 128 : dc * 512 + (r + 1) * 128],
                            wvt[:, dc * 512 : (dc + 1) * 512], dc == 0,
                            dc == DC - 1)
                    rc = grp * 4 + r
                    nc.vector.tensor_copy(kv_v[:, rc * 512 : (rc + 1) * 512],
                                          ps_v[:])

        # ===== stage 3: attention =====
        with (
            tc.tile_pool(name="s3", bufs=2) as s3,
            tc.tile_pool(name="s3ps", bufs=2, space="PSUM") as s3ps,
        ):
            for grp in range(2):
                for h in range(HQ):
                    g = h // 4
                    expb = s3.tile([128, 8 * 128], BF16, tag="expb")
                    for kc in range(8):
                        ps_s = s3ps.tile([128, 128], F32, tag="ps_s")
                        _mm(nc, ps_s[:],
                            kv_kT[:, g * T + grp * 1024 + kc * 128
                                  : g * T + grp * 1024 + (kc + 1) * 128],
                            qT[:, h * TOWN + grp * 128
                               : h * TOWN + (grp + 1) * 128],
                            True, True)
                        ex = s3.tile([128, 128], BF16, tag="ex")
                        nc.scalar.activation(ex[:], ps_s[:], AOT.Exp, scale=ISR)
                        nc.vector.tensor_tensor(
                            expb[:, kc * 128 : (kc + 1) * 128], ex[:],
                            maskt[:, (grp * 8 + kc) * 128
                                  : (grp * 8 + kc + 1) * 128], MUL)
                    ps_den = s3ps.tile([1, 128], F32, tag="ps_den")
                    for kc in range(8):
                        _mm(nc, ps_den[:], cst["ones_col_b"][:],
                            expb[:, kc * 128 : (kc + 1) * 128], kc == 0, kc == 7)
                    denr = s3.tile([1, 128], F32, tag="denr")
                    nc.vector.reciprocal(denr[:], ps_den[:])
                    denb = s3.tile([1, 128], BF16, tag="denb")
                    nc.vector.tensor_copy(denb[:], denr[:])
                    ps_bc = s3ps.tile([128, 128], F32, tag="ps_dbc")
                    _mm(nc, ps_bc[:], cst["ones_row_b"][:], denb[:], True, True)
                    ps_o = s3ps.tile([128, 128], F32, tag="ps_o")
                    for kc in range(8):
                        at = s3.tile([128, 128], BF16, tag="at")
                        nc.vector.tensor_tensor(
                            at[:], expb[:, kc * 128 : (kc + 1) * 128], ps_bc[:],
                            MUL)
                        rc = grp * 8 + kc
                        _mm(nc, ps_o[:],
                            kv_v[:, rc * 512 + g * 128 : rc * 512 + (g + 1) * 128],
                            at[:], kc == 0, kc == 7)
                    nc.vector.tensor_copy(
                        oTh[:, h * TOWN + grp * 128 : h * TOWN + (grp + 1) * 128],
                        ps_o[:])

    # ===== stage 4: o-proj + residual + hnT + AllGather =====
    with (
        tc.tile_pool(name="s4", bufs=2) as s4,
        tc.tile_pool(name="s4h", bufs=1) as s4h,
        tc.tile_pool(name="s4w", bufs=3) as s4w,
        tc.tile_pool(name="s4ps", bufs=2, space="PSUM") as s4ps,
    ):
        hnT = s4h.tile([128, DC * TOWN], F32R, tag="hnT")
        for grp in range(2):
            for dg in range(4):
                ps_op = s4ps.tile([128, 512], F32, tag="ps_op")
                for h in range(HQ):
                    wot = s4w.tile([128, 512], F32R, tag="wot")
                    nc.sync.dma_start(
                        out=wot[:],
                        in_=io["wo"][h * 128 : (h + 1) * 128,
                                     dg * 512 : (dg + 1) * 512])
                    _mm(nc, ps_op[:],
                        oTh[:, h * TOWN + grp * 128 : h * TOWN + (grp + 1) * 128],
                        wot[:], h == 0, h == HQ - 1)
                xo = s4.tile([128, 512], F32, tag="xo")
                nc.sync.dma_start(
                    out=xo[:],
                    in_=io["x_own"][grp * 128 : (grp + 1) * 128,
                                    dg * 512 : (dg + 1) * 512])
                nc.vector.tensor_tensor(
                    xloc[:, grp * D + dg * 512 : grp * D + (dg + 1) * 512],
                    ps_op[:], xo[:], ADD)
            sq = s4.tile([128, D], F32, tag="sq4")
            s2acc = s4.tile([128, 1], F32, tag="s2acc")
            nc.scalar.activation(sq[:], xloc[:, grp * D : (grp + 1) * D],
                                 mybir.ActivationFunctionType.Square,
                                 accum_out=s2acc[:])
            sr = s4.tile([128, 1], F32, tag="sr")
            nc.scalar.activation(sr[:], s2acc[:], mybir.ActivationFunctionType.Sqrt,
                                 scale=1.0 / D, bias=cst["epsc"][:])
            rr = s4.tile([128, 1], F32, tag="rr")
            nc.vector.reciprocal(rr[:], sr[:])
            hs = s4.tile([128, D], F32, tag="hs")
            nc.vector.tensor_scalar_mul(hs[:], xloc[:, grp * D : (grp + 1) * D],
                                        rr[:])
            for dc in range(DC):
                ps_t = s4ps.tile([128, 128], F32, tag="ps_t4")
                nc.tensor.transpose(ps_t[:], hs[:, dc * 128 : (dc + 1) * 128],
                                    cst["identt"][:])
                nc.vector.tensor_scalar_mul(
                    hnT[:, dc * TOWN + grp * 128 : dc * TOWN + (grp + 1) * 128],
                    ps_t[:], cst["fwt"][:, dc : dc + 1])
        for dc in range(DC):
            nc.sync.dma_start(
                out=dram["ag_in"][dc * 128 : (dc + 1) * 128, :].bitcast(F32R),
                in_=hnT[:, dc * TOWN : (dc + 1) * TOWN])
        nc.gpsimd.collective_compute(
            "AllGather", mybir.AluOpType.bypass,
            replica_groups=[list(range(NCORES))],
            ins=[dram["ag_in"].opt()], outs=[dram["ag_out"].opt()])

    # ===== stage 5: router + MoE + ReduceScatter =====
    with (
        tc.tile_pool(name="s5", bufs=2) as s5,
        tc.tile_pool(name="s5h", bufs=1) as s5h,
        tc.tile_pool(name="s5w", bufs=2) as s5w,
        tc.tile_pool(name="s5d", bufs=3) as s5d,
        tc.tile_pool(name="s5y", bufs=4) as s5y,
        tc.tile_pool(name="s5ps", bufs=2, space="PSUM") as s5ps,
        tc.tile_pool(name="s5psY", bufs=4, space="PSUM") as s5psY,
    ):
        acc_is = s5h.tile([128, E], F32, tag="acc_is")
        nc.gpsimd.memset(acc_is[:], 0.0)
        acc_p = s5h.tile([128, E], F32, tag="acc_p")
        nc.gpsimd.memset(acc_p[:], 0.0)
        wd_all = s5h.tile([128, 16], F32, tag="wd_all")
        rwt = s5h.tile([128, DC * E], F32R, tag="rwt")
        for dc in range(DC):
            nc.sync.dma_start(out=rwt[:, dc * E : (dc + 1) * E],
                              in_=io["router_w"][dc * 128 : (dc + 1) * 128, :])

        for tg in range(4):
            hn512 = s5h.tile([128, DC * 512], F32R, tag="hn512")
            for half in range(2):
                sec = tg * 2 + half
                for dc in range(DC):
                    nc.sync.dma_start(
                        out=hn512[:, dc * 512 + half * 256
                                  : dc * 512 + half * 256 + 256],
                        in_=dram["ag_out"][sec * D + dc * 128
                                           : sec * D + (dc + 1) * 128,
                                           :].bitcast(F32R))
            # ---- router / top-2 / aux ----
            for sub in range(4):
                ps_r = s5ps.tile([128, E], F32, tag="ps_g")
                for dc in range(DC):
                    _mm(nc, ps_r[:],
                        hn512[:, dc * 512 + sub * 128 : dc * 512 + (sub + 1) * 128],
                        rwt[:, dc * E : (dc + 1) * E], dc == 0, dc == DC - 1)
                nmax = s5.tile([128, 1], F32, tag="nmax")
                nc.vector.reduce_max(nmax[:], ps_r[:], mybir.AxisListType.X,
                                     negate=True)
                pe = s5.tile([128, E], F32, tag="pe")
                nc.scalar.activation(pe[:], ps_r[:], AOT.Exp, bias=nmax[:])
                psum_ = s5.tile([128, 1], F32, tag="psum_")
                nc.vector.reduce_sum(psum_[:], pe[:], mybir.AxisListType.X)
                prec = s5.tile([128, 1], F32, tag="prec")
                nc.vector.reciprocal(prec[:], psum_[:])
                probs = s5.tile([128, E], F32, tag="probs")
                nc.vector.tensor_scalar_mul(probs[:], pe[:], prec[:])
                m1 = s5.tile([128, 1], F32, tag="m1")
                nc.vector.reduce_max(m1[:], probs[:], mybir.AxisListType.X)
                is1 = s5.tile([128, E], F32, tag="is1")
                nc.vector.tensor_scalar(is1[:], probs[:], m1[:], None,
                                        mybir.AluOpType.is_ge)
                pm2 = s5.tile([128, E], F32, tag="pm2")
                nc.vector.scalar_tensor_tensor(
                    pm2[:], probs[:], m1[:], probs[:], mybir.AluOpType.is_lt, MUL)
                m2 = s5.tile([128, 1], F32, tag="m2")
                nc.vector.reduce_max(m2[:], pm2[:], mybir.AxisListType.X)
                is2 = s5.tile([128, E], F32, tag="is2")
                nc.vector.tensor_scalar(is2[:], pm2[:], m2[:], None,
                                        mybir.AluOpType.is_ge)
                istop = s5.tile([128, E], F32, tag="istop")
                nc.vector.tensor_tensor(istop[:], is1[:], is2[:], ADD)
                m12 = s5.tile([128, 1], F32, tag="m12")
                nc.vector.tensor_tensor(m12[:], m1[:], m2[:], ADD)
                r12 = s5.tile([128, 1], F32, tag="r12")
                nc.vector.reciprocal(r12[:], m12[:])
                wdense = s5.tile([128, E], F32, tag="wdense")
                nc.vector.scalar_tensor_tensor(
                    wdense[:], probs[:], r12[:], istop[:], MUL, MUL)
                wsel = s5.tile([128, E], F32, tag="wsel")
                nc.vector.tensor_tensor(wsel[:], wdense[:], cst["eselt"][:], MUL)
                nc.vector.reduce_sum(
                    wd_all[:, tg * 4 + sub : tg * 4 + sub + 1], wsel[:],
                    mybir.AxisListType.X)
                nc.vector.tensor_tensor(acc_is[:], acc_is[:], istop[:], ADD)
                nc.vector.tensor_tensor(acc_p[:], acc_p[:], probs[:], ADD)

            # ---- routed z ----
            zt = s5h.tile([128, FC * 512], F32R, tag="zt")
            for fc in range(FC):
                wgt = s5w.tile([128, DC * 128], F32R, tag="wgt")
                for dc in range(DC):
                    nc.sync.dma_start(
                        out=wgt[:, dc * 128 : (dc + 1) * 128],
                        in_=io["wg"][dc * 128 : (dc + 1) * 128,
                                     fc * 128 : (fc + 1) * 128])
                wut = s5w.tile([128, DC * 128], F32R, tag="wut")
                for dc in range(DC):
                    nc.sync.dma_start(
                        out=wut[:, dc * 128 : (dc + 1) * 128],
                        in_=io["wu"][dc * 128 : (dc + 1) * 128,
                                     fc * 128 : (fc + 1) * 128])
                ps_g = s5ps.tile([128, 512], F32, tag="ps_g")
                for dc in range(DC):
                    _mm(nc, ps_g[:], wgt[:, dc * 128 : (dc + 1) * 128],
                        hn512[:, dc * 512 : (dc + 1) * 512], dc == 0, dc == DC - 1)
                ps_u = s5ps.tile([128, 512], F32, tag="ps_u")
                for dc in range(DC):
                    _mm(nc, ps_u[:], wut[:, dc * 128 : (dc + 1) * 128],
                        hn512[:, dc * 512 : (dc + 1) * 512], dc == 0, dc == DC - 1)
                sg = s5.tile([128, 512], F32, tag="sg")
                nc.scalar.activation(sg[:], ps_g[:], AOT.Silu)
                nc.vector.tensor_tensor(zt[:, fc * 512 : (fc + 1) * 512],
                                        sg[:], ps_u[:], MUL)
            # ---- shared z ----
            zs = s5h.tile([128, len(FSC) * 512], F32R, tag="zs")
            foff = 0
            for i, fw in enumerate(FSC):
                wgt = s5w.tile([128, DC * 128], F32R, tag="wgt")
                for dc in range(DC):
                    nc.sync.dma_start(
                        out=wgt[:, dc * 128 : dc * 128 + fw],
                        in_=io["wsg"][dc * 128 : (dc + 1) * 128, foff : foff + fw])
                wut = s5w.tile([128, DC * 128], F32R, tag="wut")
                for dc in range(DC):
                    nc.sync.dma_start(
                        out=wut[:, dc * 128 : dc * 128 + fw],
                        in_=io["wsu"][dc * 128 : (dc + 1) * 128, foff : foff + fw])
                ps_g = s5ps.tile([128, 512], F32, tag="ps_g")
                for dc in range(DC):
                    _mm(nc, ps_g[:fw, :], wgt[:, dc * 128 : dc * 128 + fw],
                        hn512[:, dc * 512 : (dc + 1) * 512], dc == 0, dc == DC - 1)
                ps_u = s5ps.tile([128, 512], F32, tag="ps_u")
                for dc in range(DC):
                    _mm(nc, ps_u[:fw, :], wut[:, dc * 128 : dc * 128 + fw],
                        hn512[:, dc * 512 : (dc + 1) * 512], dc == 0, dc == DC - 1)
                sg = s5.tile([128, 512], F32, tag="sg")
                nc.scalar.activation(sg[:fw, :], ps_g[:fw, :], AOT.Silu)
                nc.vector.tensor_tensor(zs[:fw, i * 512 : (i + 1) * 512],
                                        sg[:fw, :], ps_u[:fw, :], MUL)
                foff += fw

            # ---- down-proj (dg outer so each weight tile is read once) ----
            for dg in range(4):
                pys = []
                for _pyi in range(4):
                    py = s5psY.tile([128, 512], F32, tag="ps_y")
                    pys.append(py)
                for fc in range(FC):
                    wdt = s5d.tile([128, 512], F32R, tag="wdt")
                    nc.sync.dma_start(
                        out=wdt[:],
                        in_=io["wd"][fc * 128 : (fc + 1) * 128,
                                     dg * 512 : (dg + 1) * 512])
                    for sub in range(4):
                        _mm(nc, pys[sub][:],
                            zt[:, fc * 512 + sub * 128 : fc * 512 + (sub + 1) * 128],
                            wdt[:], fc == 0, fc == FC - 1)
                yscs = []
                for sub in range(4):
                    ysc = s5y.tile([128, 512], F32, tag="ysc")
                    nc.vector.tensor_scalar_mul(
                        ysc[:], pys[sub][:],
                        wd_all[:, tg * 4 + sub : tg * 4 + sub + 1])
                    yscs.append(ysc)
                pshs = []
                for _pyi in range(4):
                    psh = s5psY.tile([128, 512], F32, tag="ps_y")
                    pshs.append(psh)
                foff = 0
                for i, fw in enumerate(FSC):
                    wsdt = s5d.tile([128, 512], F32R, tag="wdt")
                    nc.sync.dma_start(
                        out=wsdt[:fw, :],
                        in_=io["wsd"][foff : foff + fw, dg * 512 : (dg + 1) * 512])
                    for sub in range(4):
                        _mm(nc, pshs[sub][:],
                            zs[:fw, i * 512 + sub * 128 : i * 512 + (sub + 1) * 128],
                            wsdt[:fw, :], i == 0, i == len(FSC) - 1)
                    foff += fw
                for sub in range(4):
                    om = s5.tile([128, 512], F32, tag="om")
                    nc.vector.tensor_tensor(om[:], yscs[sub][:], pshs[sub][:], ADD)
                    nc.sync.dma_start(
                        out=dram["rs_in"][tg * 512 + sub * 128
                                          : tg * 512 + (sub + 1) * 128,
                                          dg * 512 : (dg + 1) * 512],
                        in_=om[:])

        # ---- aux loss ----
        acc_is_r = s5.tile([128, E], F32R, tag="acc_is_r")
        nc.vector.tensor_copy(acc_is_r[:], acc_is[:])
        acc_p_r = s5.tile([128, E], F32R, tag="acc_p_r")
        nc.vector.tensor_copy(acc_p_r[:], acc_p[:])
        ps_a = s5ps.tile([1, E], F32, tag="ps_g")
        _mm(nc, ps_a[:], ones_col_f[:], acc_is_r[:], True, True)
        fa = s5.tile([1, E], F32, tag="fa")
        nc.vector.tensor_copy(fa[:], ps_a[:])
        ps_a2 = s5ps.tile([1, E], F32, tag="ps_g")
        _mm(nc, ps_a2[:], ones_col_f[:], acc_p_r[:], True, True)
        fp_ = s5.tile([1, E], F32, tag="fp_")
        nc.vector.tensor_tensor(fp_[:], fa[:], ps_a2[:], MUL)
        auxs = s5.tile([1, 1], F32, tag="auxs")
        nc.vector.reduce_sum(auxs[:], fp_[:], mybir.AxisListType.X)
        auxo = s5.tile([1, 1], F32, tag="auxo")
        nc.scalar.mul(auxo[:], auxs[:], float(E) / (T * TOPK * T))
        nc.sync.dma_start(out=io["aux"][:], in_=auxo[:])

        nc.gpsimd.collective_compute(
            "ReduceScatter", mybir.AluOpType.add,
            replica_groups=[list(range(NCORES))],
            ins=[dram["rs_in"].opt()], outs=[dram["rs_out"].opt()])

    # ===== stage 6: final residual =====
    with tc.tile_pool(name="s6", bufs=2) as s6:
        for grp in range(2):
            mt = s6.tile([128, D], F32, tag="mt")
            nc.sync.dma_start(out=mt[:],
                              in_=dram["rs_out"][grp * 128 : (grp + 1) * 128, :])
            ot = s6.tile([128, D], F32, tag="ot")
            nc.vector.tensor_tensor(ot[:], xloc[:, grp * D : (grp + 1) * D],
                                    mt[:], ADD)
            nc.sync.dma_start(out=io["out_sh"][grp * 128 : (grp + 1) * 128, :],
                              in_=ot[:])


def _split_multi_waits(nc):
    """walrus in this container supports very few sync-wait slots per
    instruction; hoist extra waits onto preceding same-engine NoOps."""
    for bb in nc.m.functions[0].blocks:
        insts = bb.instructions
        new_list = []
        changed = False
        for inst in insts:
            si = inst.sync_info
            if si is not None and len(si.on_wait) > 1:
                waits = list(si.on_wait)
                for j, w in enumerate(waits[:-1]):
                    new_list.append(mybir.InstNoOp(
                        name=f"{inst.name}_ws{j}", engine=inst.engine,
                        sync_info=mybir.SyncInfo(on_wait=[w], on_update=[]),
                        bass_nofuse=True))
                inst.sync_info = mybir.SyncInfo(
                    on_wait=[waits[-1]], on_update=list(si.on_update))
                changed = True
            new_list.append(inst)
        if changed:
            insts[:] = new_list


# ================= host-side sharding / unsharding =================

def make_in_maps(x, rope_cos, rope_sin, attn_norm_w, ffn_norm_w,
                 wq, bq, wk, bk, wv, bv, wo, router_w,
                 w_gate, w_up, w_down, ws_gate, ws_up, ws_down):
    import ml_dtypes
    f = lambda a: np.ascontiguousarray(np.asarray(a, np.float32))
    x = f(x)
    cosT_t = np.concatenate([f(rope_cos).T, f(rope_cos).T], axis=1)
    sinT_t = np.concatenate([f(rope_sin).T, f(rope_sin).T], axis=1)
    xT_b = [np.ascontiguousarray(x[b].T) for b in range(B)]
    xT_full = np.ascontiguousarray(np.concatenate(xT_b, axis=1))
    ident = np.eye(128, dtype=np.float32)
    tri = np.triu(np.ones((128, 128), np.float32))  # visible: krow <= qrow

    in_maps = []
    for c in range(NCORES):
        b0, b1 = c, 7 - c
        x_own = np.concatenate(
            [x[0][b0 * 128:(b0 + 1) * 128], x[1][b1 * 128:(b1 + 1) * 128]], 0)
        xT_own = np.concatenate(
            [xT_b[0][:, b0 * 128:(b0 + 1) * 128],
             xT_b[1][:, b1 * 128:(b1 + 1) * 128]], 1)
        cosT_own = np.concatenate(
            [cosT_t[:, b0 * 128:(b0 + 1) * 128],
             cosT_t[:, b1 * 128:(b1 + 1) * 128]], 1)
        sinT_own = np.concatenate(
            [sinT_t[:, b0 * 128:(b0 + 1) * 128],
             sinT_t[:, b1 * 128:(b1 + 1) * 128]], 1)
        am = np.zeros((2, 8, 128, 128), np.float32)
        for kc in range(8):
            if kc < b0:
                am[0, kc] = 1.0
            elif kc == b0:
                am[0, kc] = tri
            if kc < b1:
                am[1, kc] = 1.0
            elif kc == b1:
                am[1, kc] = tri
        am = am.reshape(16 * 128, 128).astype(ml_dtypes.bfloat16)
        es = np.zeros((128, E), np.float32)
        es[:, c] = 1.0
        in_maps.append(dict(
            xT=xT_full, cosT=cosT_t, sinT=sinT_t,
            attn_w=f(attn_norm_w).reshape(D, 1),
            ffn_w=f(ffn_norm_w).reshape(D, 1),
            wq=f(wq), wk=f(wk), wv=f(wv), wo=f(wo), router_w=f(router_w),
            identity=ident,
            x_own=np.ascontiguousarray(x_own),
            xT_own=np.ascontiguousarray(xT_own),
            cosT_own=np.ascontiguousarray(cosT_own),
            sinT_own=np.ascontiguousarray(sinT_own),
            amask=am, esel=es,
            wg=f(w_gate[c]), wu=f(w_up[c]), wd=f(w_down[c]),
            wsg=f(ws_gate[:, c * FSS:(c + 1) * FSS]),
            wsu=f(ws_up[:, c * FSS:(c + 1) * FSS]),
            wsd=f(ws_down[c * FSS:(c + 1) * FSS, :]),
        ))
    return in_maps


def unshard(results):
    out = np.zeros((B, S, D), np.float32)
    nk = np.zeros((B, S, HQ, HD), np.float32)
    nv = np.zeros((B, S, HQ, HD), np.float32)
    for c in range(NCORES):
        r = results[c]
        b0, b1 = c, 7 - c
        out[0, b0 * 128:(b0 + 1) * 128] = r["out_sh"][:128]
        out[1, b1 * 128:(b1 + 1) * 128] = r["out_sh"][128:]
        nk[0, b0 * 128:(b0 + 1) * 128] = r["newk"][:128].reshape(128, HQ, HD)
        nk[1, b1 * 128:(b1 + 1) * 128] = r["newk"][128:].reshape(128, HQ, HD)
        nv[0, b0 * 128:(b0 + 1) * 128] = r["newv"][:128].reshape(128, HQ, HD)
        nv[1, b1 * 128:(b1 + 1) * 128] = r["newv"][128:].reshape(128, HQ, HD)
    aux_loss = np.float32(results[0]["aux"][0, 0])
    return out, (nk, nv), aux_loss


_CACHED = {}


def kernel(**inputs):
    if "nc" not in _CACHED:
        _CACHED["nc"] = build_bass(repeats=1)
    nc = _CACHED["nc"]
    in_maps = make_in_maps(**inputs)
    from concourse.bass_utils import run_bass_kernel_spmd
    res = run_bass_kernel_spmd(nc, in_maps, list(range(NCORES)))
    return unshard(res.results)
